# revision 5
# baseline (speedup 1.0000x reference)
"""Trainium2 Bass kernel for complex-valued multi-head attention with key masking.

Problem (hardcoded shapes): B=4, Nq=Nk=1024, R=256, NH=8, DK=DV=64.
  Q,K,V complex [B,N,R] (given as _real/_imag f32 pairs), complex weights
  WQ/WK/WV [512,256], WO [256,512], boolean key mask [B,Nk].
  out = complex MHA(Q,K,V) with softmax over |scores| restricted to valid keys.

Sharding: 8 cores = (batch b in 0..3) x (head-group hg in 0..1, 4 heads each).
Each core computes its batch's attention for its 4 heads plus the partial
output projection; the host sums the two head-group partials per batch.

Layout: everything transposed (channels on partitions, sequence on free dim).
Complex arithmetic is folded into matmuls by stacking real/imag along the
128-partition contraction dim.  Scores are computed TRANSPOSED (S^T[k,q]:
lhsT = K-projection block, rhs = Q-projection), so the softmaxed weights are
already in the [k,q] layout the attention matmul needs -- no DRAM bounce and
no DMA transpose.  The imaginary part uses Kb = swap-halves(Ka) with one half
negated.  Softmax: |s|^2 via two fused DVE ops, sqrt+exp on ACT (phase-
grouped per head to amortize table loads), pad keys are killed with a
per-partition bias of -60 on the exp (e^-60 ~ 0).  Denominator = ones-vector
matmul on the PE (partition reduction), reciprocal on DVE, broadcast across
partitions via a small DRAM-bounce DMA, applied during the PSUM->SBUF drain
of attention one head-iteration later (hides the DMA latency).

Pipeline (iteration h): PE runs scores(h), attn+den(h-1), projections(h+1);
ACT runs exp(h-1) then sqrt(h) (one sqrt + one exp table load per head);
DVE runs the |s|^2 drains (the pace-setter), normalization mult of h-2 and
reciprocal of h-1.  All inputs are host-packed in device layout so each
tensor is one contiguous DMA, issued across four engine queues.
"""

import numpy as np
import ml_dtypes

B, NQ, NK, R = 4, 1024, 1024, 256
NH, DK, DV = 8, 64, 64
NCORES = 8
NHL = 4          # heads per core
F32MIN_PAD = 640  # minimum padded key count (keys padded to a multiple of 128)

_BF16 = ml_dtypes.bfloat16

# ----------------------------------------------------------------------------
# custom DVE ops (registered at import into concourse's op table)
# ----------------------------------------------------------------------------
_OPS = {}


def _register_custom_ops():
    if _OPS:
        return
    import concourse.dve_ops as dom
    from concourse.dve_ops import DveOp
    from concourse.dve_spec import Spec, Src0, Src1, C0, sq, lower, _has_src1
    from concourse.dve_uop import DveOpSpec

    def make(name, spec):
        if name in dom._SUB_OPCODE_FOR_NAME:
            _OPS[name] = next(o for o in dom.OPS if o.name == name)
            return
        row = dom._CUSTOM_DVE_ROW_BASE + len(dom.OPS)
        assert row < 0x20, "custom DVE row overflow"
        shas = {}
        for ver in ("v3", "v4"):
            tmp = DveOpSpec(name=name, opcode=row, uops=lower(spec, ver=ver),
                            rd1_en=_has_src1(spec))
            shas[ver] = tmp.sha(ver)
        op = DveOp(name, spec, subdim=False, uops_sha=shas)
        dom.OPS.append(op)
        dom._SUB_OPCODE_FOR_NAME[name] = row
        dom.CUSTOM_DVE_SPECS[name] = spec
        _OPS[name] = op

    # t = (in0*s0)^2          (drains+squares one score tile from PSUM)
    make("CMHA_SQSC", Spec(
        body=sq(Src0 * C0),
        reference=lambda in0, in1, s0, s1, imm2: (in0.astype(np.float32) * s0) ** 2,
    ))
    # v = (in0*s0)^2 + in1 + s1   (second square, accumulate |s|^2; s1 is a
    # tiny epsilon so sqrt never sees an exact 0)
    from concourse.dve_spec import C1
    make("CMHA_SQADD", Spec(
        body=sq(Src0 * C0) + Src1 + C1,
        reference=lambda in0, in1, s0, s1, imm2: (in0.astype(np.float32) * s0) ** 2
        + in1.astype(np.float32) + s1,
    ))


# ----------------------------------------------------------------------------
# device program
# ----------------------------------------------------------------------------
_BUILD_CACHE = {}


def _build(nkp):
    """Build + compile the SPMD device program for padded key count nkp."""
    if nkp in _BUILD_CACHE:
        return _BUILD_CACHE[nkp]
    _register_custom_ops()
    import concourse.bass as bass
    import concourse.bacc as bacc
    import concourse.mybir as mybir
    import concourse.tile as tile
    from contextlib import ExitStack

    F32 = mybir.dt.float32
    BF16 = mybir.dt.bfloat16
    AF = mybir.ActivationFunctionType
    assert nkp % 128 == 0
    KB = nkp // 128
    kchunks = [(o, min(512, nkp - o)) for o in range(0, nkp, 512)]

    nc = bacc.Bacc("TRN2", target_bir_lowering=False, debug=False,
                   num_devices=NCORES)

    qt = nc.dram_tensor("qt", [128, 4 * NQ], BF16, kind="ExternalInput").ap()
    kt = nc.dram_tensor("kt", [128, 4 * nkp], BF16, kind="ExternalInput").ap()
    vt = nc.dram_tensor("vt", [128, 4 * nkp], BF16, kind="ExternalInput").ap()
    wq = nc.dram_tensor("wq", [128, NHL * 512], BF16, kind="ExternalInput").ap()
    wk = nc.dram_tensor("wk", [128, NHL * 512], BF16, kind="ExternalInput").ap()
    wv = nc.dram_tensor("wv", [128, 4 * 512], BF16, kind="ExternalInput").ap()
    wo = nc.dram_tensor("wo", [128, NHL * 512], BF16, kind="ExternalInput").ap()
    padb = nc.dram_tensor("padb", [128, KB], F32, kind="ExternalInput").ap()
    outr = nc.dram_tensor("outr", [256, NQ], F32, kind="ExternalOutput").ap()
    outi = nc.dram_tensor("outi", [256, NQ], F32, kind="ExternalOutput").ap()

    sqsc = _OPS["CMHA_SQSC"]
    sqadd = _OPS["CMHA_SQADD"]

    with tile.TileContext(nc) as tc, ExitStack() as ctx:
        const = ctx.enter_context(tc.tile_pool(name="const", bufs=1))
        # PSUM: scores/proj/VK/WO ring (2x [128,1024] = 4 banks) +
        # attention accumulator (2 banks) + denominator (2 banks) = 8 banks.
        psp = ctx.enter_context(tc.tile_pool(name="psp", bufs=2, space="PSUM"))
        acc = ctx.enter_context(tc.tile_pool(name="acc", bufs=1, space="PSUM"))
        prj = ctx.enter_context(tc.tile_pool(name="prj", bufs=1))
        smv = ctx.enter_context(tc.tile_pool(name="smv", bufs=4))
        smx = ctx.enter_context(tc.tile_pool(name="smx", bufs=12))
        esb = ctx.enter_context(tc.tile_pool(name="esb", bufs=7))
        nrm = ctx.enter_context(tc.tile_pool(name="nrm", bufs=2))
        drp = ctx.enter_context(tc.tile_pool(name="drp", bufs=2, space="DRAM"))
        outp = ctx.enter_context(tc.tile_pool(name="outp", bufs=4))

        # ---- input loads: one contiguous DMA per tensor, spread over four
        # issue queues so the first projection can start early ---------------
        def load(eng, shape, dtype, src, tag):
            t = const.tile(shape, dtype, tag=tag, name=tag)
            eng.dma_start(t[:], src)
            return t

        wq_sb = load(nc.sync, [128, NHL * 512], BF16, wq, "wq")
        qt_sb = load(nc.sync, [128, 4 * NQ], BF16, qt, "qt")
        wk_sb = load(nc.gpsimd, [128, NHL * 512], BF16, wk, "wk")
        kt_sb = load(nc.gpsimd, [128, 4 * nkp], BF16, kt, "kt")
        wv_sb = load(nc.scalar, [128, 4 * 512], BF16, wv, "wv")
        vt_sb = load(nc.scalar, [128, 4 * nkp], BF16, vt, "vt")
        padb_sb = load(nc.scalar, [128, KB], F32, padb, "padb")
        wo_sb = load(nc.gpsimd, [128, NHL * 512], BF16, wo, "wo")

        VK = const.tile([128, 512 * KB], BF16, tag="vk", name="VK")
        ATT = [const.tile([128, NQ], BF16, tag=f"att{h}", name=f"ATT{h}")
               for h in range(NHL)]
        ones = const.tile([128, 1], BF16, tag="ones", name="ones")
        nc.vector.memset(ones[:], 1.0)

        def mm(out_ap, lhsT, rhs, start=True, stop=True):
            nc.tensor.matmul(out_ap, lhsT, rhs, start=start, stop=stop)

        # ---- projections ---------------------------------------------------
        qa_sb = [None] * NHL
        ka_sb = [None] * NHL
        kb_sb = [None] * NHL

        def emit_proj(h):
            qa_ps = psp.tile([128, 1024], F32, tag="ps", name="qa_ps")
            for qc in range(2):
                for c in range(4):
                    mm(qa_ps[:, qc * 512:(qc + 1) * 512],
                       wq_sb[:, h * 512 + c * 128:h * 512 + (c + 1) * 128],
                       qt_sb[:, c * NQ + qc * 512:c * NQ + (qc + 1) * 512],
                       c == 0, c == 3)
            qa = prj.tile([128, NQ], BF16, tag=f"qa{h}", name="qa")
            nc.any.tensor_copy(qa[:], qa_ps[:])
            qa_sb[h] = qa

            ka_ps = psp.tile([128, 1024], F32, tag="ps", name="ka_ps")
            for (o, w_) in kchunks:
                for c in range(4):
                    mm(ka_ps[:, o:o + w_],
                       wk_sb[:, h * 512 + c * 128:h * 512 + (c + 1) * 128],
                       kt_sb[:, c * nkp + o:c * nkp + o + w_], c == 0, c == 3)
            ka = prj.tile([128, nkp], BF16, tag=f"ka{h}", name="ka")
            nc.any.tensor_copy(ka[:], ka_ps[:, 0:nkp])
            ka_sb[h] = ka
            # Kb = [Kp_i^T; -Kp_r^T]: swap the halves via SBUF->SBUF DMA, then
            # negate the second half in place on the DVE.
            kb_t = prj.tile([128, nkp], BF16, tag=f"kb{h}", name="kb_t")
            nc.sync.dma_start(kb_t[0:64, :], ka[64:128, :])
            nc.sync.dma_start(kb_t[64:128, :], ka[0:64, :])
            nc.vector.tensor_scalar_mul(kb_t[64:128, :], kb_t[64:128, :], -1.0)
            kb_sb[h] = kb_t

        def emit_vk_block(kb):
            ko = kb * 128
            ps = psp.tile([128, 512], F32, tag="ps", name="vk_ps")
            for c in range(4):
                mm(ps[:, 0:512], vt_sb[:, c * nkp + ko:c * nkp + ko + 128],
                   wv_sb[:, c * 512:(c + 1) * 512], c == 0, c == 3)
            nc.any.tensor_copy(VK[:, kb * 512:(kb + 1) * 512], ps[:, 0:512])

        # ---- per-head pipeline ---------------------------------------------
        x_tiles = [None] * NHL   # sqrt outputs per head (list of KB tiles)
        e_tiles = [None] * NHL   # exp outputs per head
        att_ps_l = [None] * NHL
        den_ps_l = [None] * NHL
        rdb_l = [None] * NHL
        prev_sqrt_last = [None]
        prev_exp_last = [None]

        def emit_exp_phase(h):
            es = []
            for kb in range(KB):
                e = esb.tile([128, NQ], BF16, tag="e", name="e")
                ei = nc.scalar.activation(e[:], x_tiles[h][kb][:], AF.Exp,
                                          bias=padb_sb[:, kb:kb + 1])
                if prev_sqrt_last[0] is not None:
                    tile.add_dep_helper(ei.ins, prev_sqrt_last[0], sync=False,
                                        reason="act phase order")
                es.append(e)
                prev_exp_last[0] = ei.ins
            e_tiles[h] = es

        def emit_scores_block(h, kb):
            ko = kb * 128
            sr = psp.tile([128, 1024], F32, tag="ps", name="sr")
            for qc in range(2):
                mm(sr[:, qc * 512:(qc + 1) * 512],
                   ka_sb[h][:, ko:ko + 128],
                   qa_sb[h][:, qc * 512:(qc + 1) * 512])
            si = psp.tile([128, 1024], F32, tag="ps", name="si")
            for qc in range(2):
                mm(si[:, qc * 512:(qc + 1) * 512],
                   kb_sb[h][:, ko:ko + 128],
                   qa_sb[h][:, qc * 512:(qc + 1) * 512])
            t = smv.tile([128, NQ], BF16, tag="t", name="t")
            nc.vector._custom_dve(sqsc, out=t[:], in0=sr[:], s0=0.125)
            v = smv.tile([128, NQ], BF16, tag="v", name="v")
            nc.vector._custom_dve(sqadd, out=v[:], in0=si[:], in1=t[:],
                                  s0=0.125, s1=1e-20)
            x = smx.tile([128, NQ], BF16, tag="x", name="x")
            si_ = nc.scalar.activation(x[:], v[:], AF.Sqrt)
            if prev_exp_last[0] is not None:
                tile.add_dep_helper(si_.ins, prev_exp_last[0], sync=False,
                                    reason="act phase order")
            prev_sqrt_last[0] = si_.ins
            x_tiles[h].append(x)

        def emit_attn_den_block(h, kb):
            for qc in range(2):
                mm(att_ps_l[h][:, qc * 512:(qc + 1) * 512],
                   VK[:, kb * 512 + h * 128: kb * 512 + (h + 1) * 128],
                   e_tiles[h][kb][:, qc * 512:(qc + 1) * 512],
                   start=(kb == 0), stop=(kb == KB - 1))
            for qc in range(2):
                mm(den_ps_l[h][0:1, qc * 512:(qc + 1) * 512],
                   ones[:, 0:1],
                   e_tiles[h][kb][:, qc * 512:(qc + 1) * 512],
                   start=(kb == 0), stop=(kb == KB - 1))

        def emit_norm_recip(h):
            rden = nrm.tile([1, NQ], F32, tag="rden", name="rden")
            for qc in range(2):
                nc.vector.reciprocal_approx_fast(
                    out=rden[:, qc * 512:(qc + 1) * 512],
                    in_=den_ps_l[h][0:1, qc * 512:(qc + 1) * 512])
            rdd = drp.tile([1, NQ], F32, tag="rdd", name="rdd")
            nc.sync.dma_start(rdd[:], rden[:])
            rdb = nrm.tile([128, NQ], F32, tag="rdb", name="rdb")
            nc.sync.dma_start(rdb[:], rdd[:].to_broadcast((128, NQ)))
            rdb_l[h] = rdb

        def emit_norm_mult(h):
            for qc in range(2):
                nc.vector.tensor_mul(
                    ATT[h][:, qc * 512:(qc + 1) * 512],
                    att_ps_l[h][:, qc * 512:(qc + 1) * 512],
                    rdb_l[h][:, qc * 512:(qc + 1) * 512])

        emit_proj(0)
        for h in range(NHL):
            x_tiles[h] = []
            if h >= 2:
                emit_norm_mult(h - 2)
            if h >= 1:
                emit_exp_phase(h - 1)
                att_ps_l[h - 1] = acc.tile([128, 1024], F32, tag="attp",
                                           name="att_ps")
                den_ps_l[h - 1] = acc.tile([1, 1024], F32, tag="denp",
                                           name="den_ps")
            for kb in range(KB):
                emit_scores_block(h, kb)
                if h == 0:
                    emit_vk_block(kb)
                else:
                    emit_attn_den_block(h - 1, kb)
            if h >= 1:
                emit_norm_recip(h - 1)
            if h + 1 < NHL:
                emit_proj(h + 1)
        # tail: last head's softmax + attention + output projection
        h = NHL - 1
        emit_norm_mult(h - 1)
        emit_exp_phase(h)
        att_ps_l[h] = acc.tile([128, 1024], F32, tag="attp", name="att_ps")
        den_ps_l[h] = acc.tile([1, 1024], F32, tag="denp", name="den_ps")
        for kb in range(KB):
            emit_attn_den_block(h, kb)
        emit_norm_recip(h)
        emit_norm_mult(h)

        # ---- output projection (accumulate over heads; h=3 last so the
        # first three heads' matmuls can run while head 3 normalizes) -------
        for ri in range(2):
            for blk in range(2):
                ops_ = psp.tile([128, 1024], F32, tag="ps", name="wo_ps")
                for h in range(NHL):
                    lh = wo_sb[:, h * 512 + ri * 256 + blk * 128:
                               h * 512 + ri * 256 + (blk + 1) * 128]
                    for qc in range(2):
                        nc.tensor.matmul(
                            ops_[:, qc * 512:(qc + 1) * 512], lh,
                            ATT[h][:, qc * 512:(qc + 1) * 512],
                            start=(h == 0), stop=(h == NHL - 1))
                osb = outp.tile([128, 1024], F32, tag="osb", name="osb")
                nc.scalar.copy(osb[:], ops_[:])
                dst = outr if ri == 0 else outi
                nc.sync.dma_start(dst[blk * 128:(blk + 1) * 128, :], osb[:])

    nc.compile()
    _BUILD_CACHE[nkp] = nc
    return nc


# ----------------------------------------------------------------------------
# host-side prep / gather
# ----------------------------------------------------------------------------
def _ctile(x):
    """[4*128, N] -> [128, 4*N] device layout (chunk-major columns)."""
    n = x.shape[1]
    return x.reshape(4, 128, n).transpose(1, 0, 2).reshape(128, 4 * n)


def _prep_inputs(Q_real, Q_imag, K_real, K_imag, V_real, V_imag,
                 WQ_r, WQ_i, WK_r, WK_i, WV_r, WV_i, WO_r, WO_i, mask):
    f32 = np.float32
    mask = np.asarray(mask).astype(bool)
    cnts = mask.sum(1)
    valid = mask.any(1)
    nkp = int(max(F32MIN_PAD, ((int(cnts.max()) + 127) // 128) * 128)) if valid.any() else F32MIN_PAD
    KB = nkp // 128

    # weight stacks (shared across cores up to head-group slicing)
    A_q = np.concatenate([WQ_r.T, -WQ_i.T], 0).astype(f32)   # [512, 512]
    B_q = np.concatenate([WQ_i.T, WQ_r.T], 0).astype(f32)
    A_k = np.concatenate([WK_r.T, -WK_i.T], 0).astype(f32)
    B_k = np.concatenate([WK_i.T, WK_r.T], 0).astype(f32)
    A_v = np.concatenate([WV_r.T, -WV_i.T], 0).astype(f32)
    B_v = np.concatenate([WV_i.T, WV_r.T], 0).astype(f32)

    in_maps = []
    for core in range(NCORES):
        b, hg = core // 2, core % 2
        idx = np.flatnonzero(mask[b])
        cnt = len(idx)

        def cpad(x):  # [Nk, R] -> gathered+padded [nkp, R]
            out = np.zeros((nkp, R), f32)
            out[:cnt] = x[idx]
            return out

        qtf = np.concatenate([Q_real[b].T, Q_imag[b].T], 0)      # [512, NQ]
        ktf = np.concatenate([cpad(K_real[b]).T, cpad(K_imag[b]).T], 0)
        vtf = np.concatenate([cpad(V_real[b]).T, cpad(V_imag[b]).T], 0)

        wq_l = np.empty((NHL, 512, 128), f32)
        wk_l = np.empty((NHL, 512, 128), f32)
        wv_l = np.empty((512, 512), f32)
        wo_l = np.empty((NHL, 128, 512), f32)
        for h in range(NHL):
            g = hg * NHL + h
            gc = slice(g * DK, (g + 1) * DK)
            wq_l[h, :, 0:64] = A_q[:, gc]
            wq_l[h, :, 64:128] = B_q[:, gc]
            wk_l[h, :, 0:64] = A_k[:, gc]
            wk_l[h, :, 64:128] = B_k[:, gc]
            wv_l[:, h * 128:h * 128 + 64] = A_v[:, gc]
            wv_l[:, h * 128 + 64:(h + 1) * 128] = B_v[:, gc]
            woa = np.concatenate([WO_r[:, gc].T, -WO_i[:, gc].T], 0)  # [128, 256]
            wob = np.concatenate([WO_i[:, gc].T, WO_r[:, gc].T], 0)
            wo_l[h, :, 0:256] = woa
            wo_l[h, :, 256:512] = wob

        # device layouts: [128, chunk-major free dim], one DMA per tensor
        qt_dev = _ctile(qtf).astype(_BF16)
        kt_dev = _ctile(ktf).astype(_BF16)
        vt_dev = _ctile(vtf).astype(_BF16)
        wq_dev = np.concatenate([_ctile(wq_l[h]) for h in range(NHL)], 1).astype(_BF16)
        wk_dev = np.concatenate([_ctile(wk_l[h]) for h in range(NHL)], 1).astype(_BF16)
        wv_dev = _ctile(wv_l).astype(_BF16)
        wo_dev = np.concatenate([wo_l[h] for h in range(NHL)], 1).astype(_BF16)

        # exp bias: 0 for valid keys, -60 for pad keys (e^-60 ~ 0)
        ki = np.arange(128)[:, None] + 128 * np.arange(KB)[None, :]
        padb = np.where(ki < cnt, 0.0, -60.0).astype(f32)

        in_maps.append({
            "qt": qt_dev, "kt": kt_dev, "vt": vt_dev,
            "wq": wq_dev, "wk": wk_dev, "wv": wv_dev, "wo": wo_dev,
            "padb": padb,
        })
    return in_maps, nkp, valid


def _gather(results, valid):
    out = np.zeros((B, NQ, R), np.complex64)
    for b in range(B):
        if not valid[b]:
            continue
        r = results[2 * b]["outr"] + results[2 * b + 1]["outr"]   # [256, NQ]
        i = results[2 * b]["outi"] + results[2 * b + 1]["outi"]
        out[b] = (r + 1j * i).T
    return out


def _run(inputs, trace=False, trace_kwargs=None):
    from concourse.bass_utils import run_bass_kernel_spmd
    in_maps, nkp, valid = _prep_inputs(**inputs)
    nc = _build(nkp)
    res = run_bass_kernel_spmd(nc, in_maps, core_ids=list(range(NCORES)),
                               trace=trace, **(trace_kwargs or {}))
    return _gather(res.results, valid), res


def kernel(**inputs) -> np.ndarray:
    out, _ = _run(inputs)
    return out
